# revision 1
# baseline (speedup 1.0000x reference)
import numpy as np
import jax
import jax.numpy as jnp
from jax.sharding import Mesh, PartitionSpec as P, NamedSharding

# Hardcoded problem shapes (nn_KBRDModel): B,L = batch/seq of entity ids,
# V,D = embedding table. 8 NeuronCores, data-parallel over batch; emb and
# attention params replicated so the final user @ emb.T needs no collective.
B, L, V, D = 2048, 128, 50000, 128
N_CORES = 8

def _compute(entity_ids, entity_mask, emb, attn_a, attn_b, rec_bias):
    m = entity_mask.astype(emb.dtype)                      # [B,L]
    h = emb[entity_ids]                                    # [B,L,D] ragged gather
    e = jnp.einsum('blk,ko->blo',
                   jnp.tanh(jnp.einsum('bld,dk->blk', h, attn_a)),
                   attn_b)[..., 0]                         # [B,L]
    attn = jax.nn.sigmoid(e) * m
    user = jnp.einsum('bl,bld->bd', attn, h)               # [B,D]
    return user @ emb.T + rec_bias                         # [B,V]

_jitted = None

def kernel(**inputs) -> np.ndarray:
    global _jitted
    devs = jax.devices()[:N_CORES]
    mesh = Mesh(np.array(devs), ('x',))
    batch_sh = NamedSharding(mesh, P('x', None))
    repl = NamedSharding(mesh, P())
    if _jitted is None:
        _jitted = jax.jit(
            _compute,
            in_shardings=(batch_sh, batch_sh, repl, repl, repl, repl),
            out_shardings=batch_sh,
        )
    out = _jitted(
        jnp.asarray(inputs['entity_ids'], jnp.int32),
        jnp.asarray(inputs['entity_mask'], jnp.int32),
        jnp.asarray(inputs['emb'], jnp.float32),
        jnp.asarray(inputs['attn_a'], jnp.float32),
        jnp.asarray(inputs['attn_b'], jnp.float32),
        jnp.asarray(inputs['rec_bias'], jnp.float32),
    )
    return np.asarray(out)

